# revision 15
# baseline (speedup 1.0000x reference)
"""LIPAR segment attention kernel for TRN2, 8 NeuronCores.

Problem (hardcoded): B=4, N=4096, DIM=768, H=12 heads, DH=64, S=16 segments
of M=256 tokens. q = x@Wq (scaled, rotary), kv = x@Wkv (rotary, shared K==V).
Segment t>=1 attends to segments [t-1, t]; segment 0 attends to itself.
Output projection Wo/bo for segments >=1, Wo0/bo0 for segment 0.

Sharding: the (b h) = 48 fused axis is split 8 ways -> 6 heads per core,
and because 6 divides H=12 each core works on exactly one batch:
core i -> batch i//2, heads (i%2)*6 .. (i%2)*6+6. Each core computes a
partial output projection (its 384 feature rows of Wo/Wo0); the host sums
the two partial (768, 4096) results per batch and adds biases.

On-device layout strategy (everything "transposed", feature-on-partition):
  - host passes xT = x[b].T (768, 4096) so projections need no on-device
    transpose of x.
  - qT/kvT (dh-on-partition, token-free) come straight out of the
    projection matmuls (lhsT = weight slice, rhs = xT).
  - rotary: qrot = qT*cosT + (Pshift @ qT)*sinT_signed, where Pshift is a
    host-built 128x128 signed permutation applied with one PE matmul and
    the +-sign of rotate_half is baked into the sinT table.
  - QK^T computes S^T (keys-on-partition) so softmax denominators and the
    AV matmul need no transpose of the attention matrix; exp runs on ACT
    without max-subtraction (scores are O(1); qk scale folded into Wq on
    host).
  - kv in natural (token, dh) layout (needed as AV lhsT) is produced by
    PE-transposing rotated kvT; a ones-column appended to it makes the AV
    matmul emit softmax denominators as output row 64 for free.
  - normalization: reciprocal of denom row -> PE outer-product broadcast
    (ones64 x r) -> elementwise multiply.
  - output projection consumes the (feature, token) attention output
    directly: partial outT_proj = Wo_slice^T-free matmuls, DMA'd out as
    (768, 4096); host transposes back.
"""

import numpy as np

B, N, DIM = 4, 4096, 768
H = 12
DH = 64
S = 16
M = 256
SCALE = DH**-0.5

HPC = 6            # heads per core
FPC = HPC * DH     # 384 features per core
KC = DIM // 128    # 6 contraction chunks
NB = 8             # token blocks
TB = N // NB       # 512 tokens per block (2 segments)
NCORES = 8


def _host_tables():
    """cosT/sinT (128, N) fp32 in the 2-head-stacked transposed layout and
    the signed shift permutation (128, 128)."""
    inv_freq = 1.0 / (10000.0 ** (np.arange(0, DH, 2, dtype=np.float64) / DH))
    t = np.arange(N, dtype=np.float64)
    freqs = np.outer(inv_freq, t)            # (32, N)
    r = np.arange(128)
    fidx = r % 32
    cosT = np.cos(freqs)[fidx].astype(np.float32)            # (128, N)
    sign = np.where((r % 64) < 32, -1.0, 1.0)[:, None]
    sinT = (sign * np.sin(freqs)[fidx]).astype(np.float32)   # (128, N)
    pshift = np.zeros((128, 128), dtype=np.float32)
    for m_ in range(128):
        src = m_ + 32 if (m_ % 64) < 32 else m_ - 32
        pshift[src, m_] = 1.0
    e2 = np.zeros((2, 128), dtype=np.float32)
    e2[0, :DH] = 1.0
    e2[1, DH:] = 1.0
    return cosT, sinT, pshift, e2


def _build_nc():
    import concourse.bass as bass
    import concourse.bacc as bacc
    import concourse.tile as tile
    from concourse import mybir
    from concourse.masks import make_identity
    from contextlib import ExitStack

    f32 = mybir.dt.float32
    EXP = mybir.ActivationFunctionType.Exp

    nc = bacc.Bacc("TRN2", target_bir_lowering=False)
    xT = nc.dram_tensor("xT", [DIM, N], f32, kind="ExternalInput")
    wq = nc.dram_tensor("wq", [DIM, FPC], f32, kind="ExternalInput")
    wkv = nc.dram_tensor("wkv", [DIM, FPC], f32, kind="ExternalInput")
    wo = nc.dram_tensor("wo", [FPC, DIM], f32, kind="ExternalInput")
    wo0 = nc.dram_tensor("wo0", [FPC, DIM], f32, kind="ExternalInput")
    cosT = nc.dram_tensor("cosT", [128, N], f32, kind="ExternalInput")
    sinT = nc.dram_tensor("sinT", [128, N], f32, kind="ExternalInput")
    pshift = nc.dram_tensor("pshift", [128, 128], f32, kind="ExternalInput")
    e2d = nc.dram_tensor("e2d", [2, 128], f32, kind="ExternalInput")
    outpT = nc.dram_tensor("outpT", [DIM, N], f32, kind="ExternalOutput")

    outpT_r = outpT.rearrange("(c p) n -> p c n", p=128)

    with tile.TileContext(nc) as tc, ExitStack() as ctx:
        consts = ctx.enter_context(tc.tile_pool(name="consts", bufs=1))
        xpool = ctx.enter_context(tc.tile_pool(name="xpool", bufs=2))
        cspool = ctx.enter_context(tc.tile_pool(name="cspool", bufs=2))
        rawpool = ctx.enter_context(tc.tile_pool(name="rawpool", bufs=2))
        tmppool = ctx.enter_context(tc.tile_pool(name="tmppool", bufs=2))
        qrpool = ctx.enter_context(tc.tile_pool(name="qrpool", bufs=2))
        kvrpool = ctx.enter_context(tc.tile_pool(name="kvrpool", bufs=3))
        kvnpool = ctx.enter_context(tc.tile_pool(name="kvnpool", bufs=10))
        ptpool = ctx.enter_context(tc.tile_pool(name="ptpool", bufs=14))
        otpool = ctx.enter_context(tc.tile_pool(name="otpool", bufs=2))
        prpool = ctx.enter_context(tc.tile_pool(name="prpool", bufs=3))
        smpool = ctx.enter_context(tc.tile_pool(name="smpool", bufs=4))

        mmps = ctx.enter_context(tc.tile_pool(name="mmps", bufs=3, space="PSUM"))
        stps = ctx.enter_context(tc.tile_pool(name="stps", bufs=2, space="PSUM"))
        avps = ctx.enter_context(tc.tile_pool(name="avps", bufs=2, space="PSUM"))
        rps = ctx.enter_context(tc.tile_pool(name="rps", bufs=1, space="PSUM"))

        # resident constants
        ident = consts.tile([128, 128], f32, tag="ident")
        make_identity(nc, ident)
        # head-pair broadcast selectors: eA -> partitions 0:64, eB -> 64:128
        eA = consts.tile([1, 128], f32, tag="eA")
        nc.sync.dma_start(out=eA, in_=e2d[0:1, :])
        eB = consts.tile([1, 128], f32, tag="eB")
        nc.sync.dma_start(out=eB, in_=e2d[1:2, :])
        wq_sb = consts.tile([128, KC, FPC], f32, tag="wq_sb")
        nc.sync.dma_start(out=wq_sb, in_=wq.rearrange("(c p) m -> p c m", p=128))
        wkv_sb = consts.tile([128, KC, FPC], f32, tag="wkv_sb")
        nc.sync.dma_start(out=wkv_sb, in_=wkv.rearrange("(c p) m -> p c m", p=128))
        wo_sb = consts.tile([128, 3, DIM], f32, tag="wo_sb")
        nc.sync.dma_start(out=wo_sb, in_=wo.rearrange("(c p) m -> p c m", p=128))
        wo0_sb = consts.tile([128, 3, DIM], f32, tag="wo0_sb")
        nc.sync.dma_start(out=wo0_sb, in_=wo0.rearrange("(c p) m -> p c m", p=128))
        psh_sb = consts.tile([128, 128], f32, tag="psh_sb")
        nc.sync.dma_start(out=psh_sb, in_=pshift[:, :])

        xT_r = xT.rearrange("(c p) n -> p c n", p=128)

        kvrot_prev = None
        kvn_prev = None

        for b in range(NB):
            n0 = b * TB
            # ---- load x block and rotary tables ----
            xt = xpool.tile([128, KC, TB], f32, tag="xt")
            nc.sync.dma_start(out=xt, in_=xT_r[:, :, n0 : n0 + TB])
            cosb = cspool.tile([128, TB], f32, tag="cosb")
            nc.sync.dma_start(out=cosb, in_=cosT[:, n0 : n0 + TB])
            sinb = cspool.tile([128, TB], f32, tag="sinb")
            nc.sync.dma_start(out=sinb, in_=sinT[:, n0 : n0 + TB])

            # ---- q/kv projections + rotary (transposed layout) ----
            rots = {}
            for name, wsb in (("q", wq_sb), ("kv", wkv_sb)):
                if name == "q":
                    rot = qrpool.tile([128, 3, TB], f32, tag="qrot")
                else:
                    rot = kvrpool.tile([128, 3, TB], f32, tag="kvrot")
                for t in range(3):
                    ps = mmps.tile([128, TB], f32, tag="mmps")
                    for c in range(KC):
                        nc.tensor.matmul(
                            ps,
                            lhsT=wsb[:, c, t * 128 : (t + 1) * 128],
                            rhs=xt[:, c, :],
                            start=(c == 0),
                            stop=(c == KC - 1),
                        )
                    raw = rawpool.tile([128, TB], f32, tag=f"raw{name}{t}")
                    nc.vector.tensor_copy(raw, ps)
                    shps = mmps.tile([128, TB], f32, tag="mmps")
                    nc.tensor.matmul(shps, lhsT=psh_sb, rhs=raw, start=True, stop=True)
                    tmp1 = tmppool.tile([128, TB], f32, tag="tmp1")
                    nc.gpsimd.tensor_mul(tmp1, raw, cosb)
                    tmp2 = tmppool.tile([128, TB], f32, tag="tmp2")
                    nc.vector.tensor_mul(tmp2, shps, sinb)
                    nc.gpsimd.tensor_add(rot[:, t, :], tmp1, tmp2)
                rots[name] = rot
            qrot = rots["q"]
            kvrot = rots["kv"]

            # ---- kv natural layout via PE transpose, with ones column ----
            kvn = []
            for cc in range(4):  # 128-token chunks of the block
                kt = kvnpool.tile([128, HPC, DH + 1], f32, tag="kvn")
                for t in range(3):
                    tps = mmps.tile([128, 128], f32, tag="mmps")
                    nc.tensor.transpose(
                        tps, kvrot[:, t, cc * 128 : (cc + 1) * 128], ident
                    )
                    nc.vector.tensor_copy(
                        kt[:, 2 * t : 2 * t + 2, 0:DH],
                        tps.rearrange("p (a b) -> p a b", a=2),
                    )
                nc.vector.memset(kt[:, :, DH : DH + 1], 1.0)
                kvn.append(kt)

            # ---- attention, QK batched per key segment ----
            # key seg 2b scores feed query segs {2b, 2b+1} in one N=512 MM;
            # key seg 2b-1 (prev block kv) feeds query seg 2b; key seg 2b+1
            # feeds query seg 2b+1 (and seg 2b+2 next block).
            outT = otpool.tile([128, 3, TB], f32, tag="outT")
            for t in range(3):
                pt = [{}, {}]  # per head-in-pair: key -> (tile, width)
                for m_ in range(2):
                    r0 = m_ * DH
                    jobs = []
                    if b > 0:
                        jobs += [
                            (kvrot_prev, 256 + i * 128, 0, M, ("pm", i))
                            for i in range(2)
                        ]
                    jobs += [(kvrot, i * 128, 0, TB, ("c0", i)) for i in range(2)]
                    jobs += [
                        (kvrot, 256 + i * 128, M, M, ("c1", i)) for i in range(2)
                    ]
                    for kvr, kvcol, qoff, qw, key in jobs:
                        st = stps.tile([128, qw], f32, tag="stps")
                        nc.tensor.matmul(
                            st,
                            lhsT=kvr[r0 : r0 + DH, t, kvcol : kvcol + 128],
                            rhs=qrot[r0 : r0 + DH, t, qoff : qoff + qw],
                            start=True,
                            stop=True,
                        )
                        p = ptpool.tile([128, qw], f32, tag="pt")
                        nc.scalar.activation(p, st, EXP)
                        pt[m_][key] = p
                for sl in range(2):
                    soff = sl * M
                    avs = []
                    for m_ in range(2):
                        h = 2 * t + m_
                        if sl == 0:
                            if b == 0:
                                chunks = [
                                    (pt[m_][("c0", 0)], 0, kvn[0]),
                                    (pt[m_][("c0", 1)], 0, kvn[1]),
                                ]
                            else:
                                chunks = [
                                    (pt[m_][("pm", 0)], 0, kvn_prev[2]),
                                    (pt[m_][("pm", 1)], 0, kvn_prev[3]),
                                    (pt[m_][("c0", 0)], 0, kvn[0]),
                                    (pt[m_][("c0", 1)], 0, kvn[1]),
                                ]
                        else:
                            chunks = [
                                (pt[m_][("c0", 0)], M, kvn[0]),
                                (pt[m_][("c0", 1)], M, kvn[1]),
                                (pt[m_][("c1", 0)], 0, kvn[2]),
                                (pt[m_][("c1", 1)], 0, kvn[3]),
                            ]
                        av = avps.tile([DH + 1, M], f32, tag="avps")
                        for ci, (p_, poff, kvn_t) in enumerate(chunks):
                            nc.tensor.matmul(
                                av,
                                lhsT=kvn_t[:, h, :],
                                rhs=p_[:, poff : poff + M],
                                start=(ci == 0),
                                stop=(ci == len(chunks) - 1),
                            )
                        avs.append(av)
                    # pair-packed normalization: broadcast both heads' recips
                    # into one (128, M) PSUM tile via two K=1 matmuls
                    rsbA = smpool.tile([1, M], f32, tag="rsbA")
                    nc.vector.reciprocal(rsbA, avs[0][DH : DH + 1, :])
                    rsbB = smpool.tile([1, M], f32, tag="rsbB")
                    nc.vector.reciprocal(rsbB, avs[1][DH : DH + 1, :])
                    rp = rps.tile([128, M], f32, tag="rps")
                    nc.tensor.matmul(rp, lhsT=eA, rhs=rsbA, start=True, stop=False)
                    nc.tensor.matmul(rp, lhsT=eB, rhs=rsbB, start=False, stop=True)
                    rbc = smpool.tile([128, M], f32, tag="rbc")
                    nc.vector.tensor_copy(rbc, rp)
                    nc.vector.tensor_mul(
                        outT[0:DH, t, soff : soff + M], avs[0][0:DH, :], rbc[0:DH, :]
                    )
                    nc.vector.tensor_mul(
                        outT[DH:128, t, soff : soff + M],
                        avs[1][0:DH, :],
                        rbc[DH:128, :],
                    )

            # ---- output projection (partial: this core's 384 features) ----
            if b == 0:
                ranges = [(0, M, wo0_sb), (M, TB, wo_sb)]
            else:
                ranges = [(0, TB, wo_sb)]
            for oc in range(6):
                pps = mmps.tile([128, TB], f32, tag="mmps")
                for (a0, a1, wsb) in ranges:
                    for t in range(3):
                        nc.tensor.matmul(
                            pps[:, a0:a1],
                            lhsT=wsb[:, t, oc * 128 : (oc + 1) * 128],
                            rhs=outT[:, t, a0:a1],
                            start=(t == 0),
                            stop=(t == 2),
                        )
                prj = prpool.tile([128, TB], f32, tag="prj")
                nc.vector.tensor_copy(prj, pps)
                nc.sync.dma_start(out=outpT_r[:, oc, n0 : n0 + TB], in_=prj)

            kvrot_prev = kvrot
            kvn_prev = kvn

    nc.compile()
    return nc


_CACHE = {}
TRACE = False


def kernel(x, Wq, Wkv, Wo, bo, Wo0, bo0):
    from concourse.bass_utils import run_bass_kernel_spmd

    x = np.asarray(x, dtype=np.float32)
    Wq = np.asarray(Wq, dtype=np.float32)
    Wkv = np.asarray(Wkv, dtype=np.float32)
    Wo = np.asarray(Wo, dtype=np.float32)
    bo = np.asarray(bo, dtype=np.float32)
    Wo0 = np.asarray(Wo0, dtype=np.float32)
    bo0 = np.asarray(bo0, dtype=np.float32)

    cosT, sinT, pshift, e2 = _host_tables()
    Wq_s = (Wq * SCALE).astype(np.float32)

    xTs = [np.ascontiguousarray(x[b_].T) for b_ in range(B)]
    in_maps = []
    for ci in range(NCORES):
        b_, hi = ci // 2, ci % 2
        fsl = slice(hi * FPC, (hi + 1) * FPC)
        in_maps.append(
            {
                "xT": xTs[b_],
                "wq": np.ascontiguousarray(Wq_s[:, fsl]),
                "wkv": np.ascontiguousarray(Wkv[:, fsl]),
                "wo": np.ascontiguousarray(Wo[fsl, :]),
                "wo0": np.ascontiguousarray(Wo0[fsl, :]),
                "cosT": cosT,
                "sinT": sinT,
                "pshift": pshift,
                "e2d": e2,
            }
        )

    if "nc" not in _CACHE:
        _CACHE["nc"] = _build_nc()
    nc = _CACHE["nc"]

    res = run_bass_kernel_spmd(
        nc, in_maps, core_ids=list(range(NCORES)), trace=TRACE
    )
    _CACHE["last"] = res
    parts = [r["outpT"] for r in res.results]

    out = np.empty((B, N, DIM), dtype=np.float32)
    bias = np.empty((N, DIM), dtype=np.float32)
    bias[:M] = bo0
    bias[M:] = bo
    for b_ in range(B):
        acc = parts[2 * b_] + parts[2 * b_ + 1]      # (768, 4096)
        out[b_] = acc.T + bias
    return out


# revision 20
# speedup vs baseline: 116.2249x; 116.2249x over previous
"""LIPAR segment attention kernel for TRN2, 8 NeuronCores.

Problem (hardcoded): B=4, N=4096, DIM=768, H=12 heads, DH=64, S=16 segments
of M=256 tokens. q = x@Wq (scaled, rotary), kv = x@Wkv (rotary, shared K==V).
Segment t>=1 attends to segments [t-1, t]; segment 0 attends to itself.
Output projection Wo/bo for segments >=1, Wo0/bo0 for segment 0.

Sharding: the (b h) = 48 fused axis is split 8 ways -> 6 heads per core,
and because 6 divides H=12 each core works on exactly one batch:
core i -> batch i//2, heads (i%2)*6 .. (i%2)*6+6. Each core computes a
partial output projection (its 384 feature rows of Wo/Wo0); the host sums
the two partial (768, 4096) results per batch and adds biases.

On-device layout strategy (everything "transposed", feature-on-partition):
  - host passes xT = x[b].T (768, 4096) so projections need no on-device
    transpose of x.
  - qT/kvT (dh-on-partition, token-free) come straight out of the
    projection matmuls (lhsT = weight slice, rhs = xT).
  - rotary: qrot = qT*cosT + (Pshift @ qT)*sinT_signed, where Pshift is a
    host-built 128x128 signed permutation applied with one PE matmul and
    the +-sign of rotate_half is baked into the sinT table.
  - QK^T computes S^T (keys-on-partition) so softmax denominators and the
    AV matmul need no transpose of the attention matrix; exp runs on ACT
    without max-subtraction (scores are O(1); qk scale folded into Wq on
    host).
  - kv in natural (token, dh) layout (needed as AV lhsT) is produced by
    PE-transposing rotated kvT; a ones-column appended to it makes the AV
    matmul emit softmax denominators as output row 64 for free.
  - normalization: reciprocal of denom row -> PE outer-product broadcast
    (ones64 x r) -> elementwise multiply.
  - output projection consumes the (feature, token) attention output
    directly: partial outT_proj = Wo_slice^T-free matmuls, DMA'd out as
    (768, 4096); host transposes back.
"""

import numpy as np

B, N, DIM = 4, 4096, 768
H = 12
DH = 64
S = 16
M = 256
SCALE = DH**-0.5

HPC = 6            # heads per core
FPC = HPC * DH     # 384 features per core
KC = DIM // 128    # 6 contraction chunks
NB = 8             # token blocks
TB = N // NB       # 512 tokens per block (2 segments)
NCORES = 8


def _host_tables():
    """cosT/sinT (128, N) fp32 in the 2-head-stacked transposed layout and
    the signed shift permutation (128, 128)."""
    inv_freq = 1.0 / (10000.0 ** (np.arange(0, DH, 2, dtype=np.float64) / DH))
    t = np.arange(N, dtype=np.float64)
    freqs = np.outer(inv_freq, t)            # (32, N)
    r = np.arange(128)
    fidx = r % 32
    cosT = np.cos(freqs)[fidx].astype(np.float32)            # (128, N)
    sign = np.where((r % 64) < 32, -1.0, 1.0)[:, None]
    sinT = (sign * np.sin(freqs)[fidx]).astype(np.float32)   # (128, N)
    pshift = np.zeros((128, 128), dtype=np.float32)
    for m_ in range(128):
        src = m_ + 32 if (m_ % 64) < 32 else m_ - 32
        pshift[src, m_] = 1.0
    e2 = np.zeros((2, 128), dtype=np.float32)
    e2[0, :DH] = 1.0
    e2[1, DH:] = 1.0
    return cosT, sinT, pshift, e2


def _build_nc():
    import concourse.bass as bass
    import concourse.bacc as bacc
    import concourse.tile as tile
    from concourse import mybir
    from concourse.masks import make_identity
    from contextlib import ExitStack

    f32 = mybir.dt.float32
    EXP = mybir.ActivationFunctionType.Exp

    nc = bacc.Bacc("TRN2", target_bir_lowering=False)
    xT = nc.dram_tensor("xT", [DIM, N], f32, kind="ExternalInput")
    wq = nc.dram_tensor("wq", [DIM, FPC], f32, kind="ExternalInput")
    wkv = nc.dram_tensor("wkv", [DIM, FPC], f32, kind="ExternalInput")
    wo = nc.dram_tensor("wo", [FPC, DIM], f32, kind="ExternalInput")
    wo0 = nc.dram_tensor("wo0", [FPC, DIM], f32, kind="ExternalInput")
    cosT = nc.dram_tensor("cosT", [128, N], f32, kind="ExternalInput")
    sinT = nc.dram_tensor("sinT", [128, N], f32, kind="ExternalInput")
    pshift = nc.dram_tensor("pshift", [128, 128], f32, kind="ExternalInput")
    e2d = nc.dram_tensor("e2d", [2, 128], f32, kind="ExternalInput")
    outpT = nc.dram_tensor("outpT", [DIM, N], f32, kind="ExternalOutput")

    outpT_r = outpT.rearrange("(c p) n -> p c n", p=128)

    with tile.TileContext(nc) as tc, ExitStack() as ctx:
        consts = ctx.enter_context(tc.tile_pool(name="consts", bufs=1))
        xpool = ctx.enter_context(tc.tile_pool(name="xpool", bufs=2))
        cspool = ctx.enter_context(tc.tile_pool(name="cspool", bufs=2))
        rawpool = ctx.enter_context(tc.tile_pool(name="rawpool", bufs=2))
        tmppool = ctx.enter_context(tc.tile_pool(name="tmppool", bufs=2))
        qrpool = ctx.enter_context(tc.tile_pool(name="qrpool", bufs=2))
        kvrpool = ctx.enter_context(tc.tile_pool(name="kvrpool", bufs=3))
        kvnpool = ctx.enter_context(tc.tile_pool(name="kvnpool", bufs=10))
        ptpool = ctx.enter_context(tc.tile_pool(name="ptpool", bufs=14))
        otpool = ctx.enter_context(tc.tile_pool(name="otpool", bufs=2))
        prpool = ctx.enter_context(tc.tile_pool(name="prpool", bufs=3))
        smpool = ctx.enter_context(tc.tile_pool(name="smpool", bufs=4))

        mmps = ctx.enter_context(tc.tile_pool(name="mmps", bufs=3, space="PSUM"))
        stps = ctx.enter_context(tc.tile_pool(name="stps", bufs=2, space="PSUM"))
        avps = ctx.enter_context(tc.tile_pool(name="avps", bufs=2, space="PSUM"))
        rps = ctx.enter_context(tc.tile_pool(name="rps", bufs=1, space="PSUM"))

        # resident constants
        ident = consts.tile([128, 128], f32, tag="ident")
        make_identity(nc, ident)
        # head-pair broadcast selectors: eA -> partitions 0:64, eB -> 64:128
        eA = consts.tile([1, 128], f32, tag="eA")
        nc.sync.dma_start(out=eA, in_=e2d[0:1, :])
        eB = consts.tile([1, 128], f32, tag="eB")
        nc.sync.dma_start(out=eB, in_=e2d[1:2, :])
        wq_sb = consts.tile([128, KC, FPC], f32, tag="wq_sb")
        nc.sync.dma_start(out=wq_sb, in_=wq.rearrange("(c p) m -> p c m", p=128))
        wkv_sb = consts.tile([128, KC, FPC], f32, tag="wkv_sb")
        nc.sync.dma_start(out=wkv_sb, in_=wkv.rearrange("(c p) m -> p c m", p=128))
        wo_sb = consts.tile([128, 3, DIM], f32, tag="wo_sb")
        nc.sync.dma_start(out=wo_sb, in_=wo.rearrange("(c p) m -> p c m", p=128))
        wo0_sb = consts.tile([128, 3, DIM], f32, tag="wo0_sb")
        nc.sync.dma_start(out=wo0_sb, in_=wo0.rearrange("(c p) m -> p c m", p=128))
        psh_sb = consts.tile([128, 128], f32, tag="psh_sb")
        nc.sync.dma_start(out=psh_sb, in_=pshift[:, :])

        xT_r = xT.rearrange("(c p) n -> p c n", p=128)

        kvrot_prev = None
        kvn_prev = None

        for b in range(NB):
            n0 = b * TB
            # ---- load x block and rotary tables ----
            xt = xpool.tile([128, KC, TB], f32, tag="xt")
            nc.sync.dma_start(out=xt, in_=xT_r[:, :, n0 : n0 + TB])
            cosb = cspool.tile([128, TB], f32, tag="cosb")
            nc.sync.dma_start(out=cosb, in_=cosT[:, n0 : n0 + TB])
            sinb = cspool.tile([128, TB], f32, tag="sinb")
            nc.sync.dma_start(out=sinb, in_=sinT[:, n0 : n0 + TB])

            # ---- q/kv projections + rotary (transposed layout) ----
            rots = {}
            for name, wsb in (("q", wq_sb), ("kv", wkv_sb)):
                if name == "q":
                    rot = qrpool.tile([128, 3, TB], f32, tag="qrot")
                else:
                    rot = kvrpool.tile([128, 3, TB], f32, tag="kvrot")
                for t in range(3):
                    ps = mmps.tile([128, TB], f32, tag="mmps")
                    for c in range(KC):
                        nc.tensor.matmul(
                            ps,
                            lhsT=wsb[:, c, t * 128 : (t + 1) * 128],
                            rhs=xt[:, c, :],
                            start=(c == 0),
                            stop=(c == KC - 1),
                        )
                    raw = rawpool.tile([128, TB], f32, tag=f"raw{name}{t}")
                    nc.vector.tensor_copy(raw, ps)
                    shps = mmps.tile([128, TB], f32, tag="mmps")
                    nc.tensor.matmul(shps, lhsT=psh_sb, rhs=raw, start=True, stop=True)
                    tmp1 = tmppool.tile([128, TB], f32, tag="tmp1")
                    nc.gpsimd.tensor_mul(tmp1, raw, cosb)
                    tmp2 = tmppool.tile([128, TB], f32, tag="tmp2")
                    nc.vector.tensor_mul(tmp2, shps, sinb)
                    nc.gpsimd.tensor_add(rot[:, t, :], tmp1, tmp2)
                rots[name] = rot
            qrot = rots["q"]
            kvrot = rots["kv"]

            # ---- kv natural layout via PE transpose, with ones column ----
            kvn = []
            for cc in range(4):  # 128-token chunks of the block
                kt = kvnpool.tile([128, HPC, DH + 1], f32, tag="kvn")
                for t in range(3):
                    tps = mmps.tile([128, 128], f32, tag="mmps")
                    nc.tensor.transpose(
                        tps, kvrot[:, t, cc * 128 : (cc + 1) * 128], ident
                    )
                    nc.vector.tensor_copy(
                        kt[:, 2 * t : 2 * t + 2, 0:DH],
                        tps.rearrange("p (a b) -> p a b", a=2),
                    )
                nc.vector.memset(kt[:, :, DH : DH + 1], 1.0)
                kvn.append(kt)

            # ---- attention, QK batched per key segment ----
            # key seg 2b scores feed query segs {2b, 2b+1} in one N=512 MM;
            # key seg 2b-1 (prev block kv) feeds query seg 2b; key seg 2b+1
            # feeds query seg 2b+1 (and seg 2b+2 next block).
            outT = otpool.tile([128, 3, TB], f32, tag="outT")
            for t in range(3):
                pt = [{}, {}]  # per head-in-pair: key -> (tile, width)
                for m_ in range(2):
                    r0 = m_ * DH
                    # each job group -> one (128, 512) S^T tile + one exp.
                    # entries: (kvr, kvcol, qoff, qw, dstcol)
                    groups = []
                    if b > 0:
                        groups.append(
                            ("pm", [(kvrot_prev, 256 + i * 128, 0, M, i * M)
                                    for i in range(2)])
                        )
                    groups.append(("c00", [(kvrot, 0, 0, TB, 0)]))
                    groups.append(("c01", [(kvrot, 128, 0, TB, 0)]))
                    groups.append(
                        ("c1", [(kvrot, 256 + i * 128, M, M, i * M)
                                for i in range(2)])
                    )
                    for key, ents in groups:
                        st = stps.tile([128, TB], f32, tag="stps")
                        for kvr, kvcol, qoff, qw, dcol in ents:
                            nc.tensor.matmul(
                                st[:, dcol : dcol + qw],
                                lhsT=kvr[r0 : r0 + DH, t, kvcol : kvcol + 128],
                                rhs=qrot[r0 : r0 + DH, t, qoff : qoff + qw],
                                start=True,
                                stop=True,
                            )
                        p = ptpool.tile([128, TB], f32, tag="pt")
                        nc.scalar.activation(p, st, EXP)
                        pt[m_][key] = p
                for sl in range(2):
                    soff = sl * M
                    avs = []
                    for m_ in range(2):
                        h = 2 * t + m_
                        if sl == 0:
                            if b == 0:
                                chunks = [
                                    (pt[m_]["c00"], 0, kvn[0]),
                                    (pt[m_]["c01"], 0, kvn[1]),
                                ]
                            else:
                                chunks = [
                                    (pt[m_]["pm"], 0, kvn_prev[2]),
                                    (pt[m_]["pm"], M, kvn_prev[3]),
                                    (pt[m_]["c00"], 0, kvn[0]),
                                    (pt[m_]["c01"], 0, kvn[1]),
                                ]
                        else:
                            chunks = [
                                (pt[m_]["c00"], M, kvn[0]),
                                (pt[m_]["c01"], M, kvn[1]),
                                (pt[m_]["c1"], 0, kvn[2]),
                                (pt[m_]["c1"], M, kvn[3]),
                            ]
                        av = avps.tile([DH + 1, M], f32, tag="avps")
                        for ci, (p_, poff, kvn_t) in enumerate(chunks):
                            nc.tensor.matmul(
                                av,
                                lhsT=kvn_t[:, h, :],
                                rhs=p_[:, poff : poff + M],
                                start=(ci == 0),
                                stop=(ci == len(chunks) - 1),
                            )
                        avs.append(av)
                    # pair-packed normalization: broadcast both heads' recips
                    # into one (128, M) PSUM tile via two K=1 matmuls
                    rsbA = smpool.tile([1, M], f32, tag="rsbA")
                    nc.vector.reciprocal(rsbA, avs[0][DH : DH + 1, :])
                    rsbB = smpool.tile([1, M], f32, tag="rsbB")
                    nc.vector.reciprocal(rsbB, avs[1][DH : DH + 1, :])
                    rp = rps.tile([128, M], f32, tag="rps")
                    nc.tensor.matmul(rp, lhsT=eA, rhs=rsbA, start=True, stop=False)
                    nc.tensor.matmul(rp, lhsT=eB, rhs=rsbB, start=False, stop=True)
                    if PSUM2:
                        mul_in = rp
                    else:
                        rbc = smpool.tile([128, M], f32, tag="rbc")
                        nc.vector.tensor_copy(rbc, rp)
                        mul_in = rbc
                    nc.vector.tensor_mul(
                        outT[0:DH, t, soff : soff + M],
                        avs[0][0:DH, :],
                        mul_in[0:DH, :],
                    )
                    nc.vector.tensor_mul(
                        outT[DH:128, t, soff : soff + M],
                        avs[1][0:DH, :],
                        mul_in[DH:128, :],
                    )

            # ---- output projection (partial: this core's 384 features) ----
            if b == 0:
                ranges = [(0, M, wo0_sb), (M, TB, wo_sb)]
            else:
                ranges = [(0, TB, wo_sb)]
            for oc in range(6):
                pps = mmps.tile([128, TB], f32, tag="mmps")
                for (a0, a1, wsb) in ranges:
                    for t in range(3):
                        nc.tensor.matmul(
                            pps[:, a0:a1],
                            lhsT=wsb[:, t, oc * 128 : (oc + 1) * 128],
                            rhs=outT[:, t, a0:a1],
                            start=(t == 0),
                            stop=(t == 2),
                        )
                prj = prpool.tile([128, TB], f32, tag="prj")
                nc.vector.tensor_copy(prj, pps)
                nc.sync.dma_start(out=outpT_r[:, oc, n0 : n0 + TB], in_=prj)

            kvrot_prev = kvrot
            kvn_prev = kvn

    nc.compile()
    return nc


_CACHE = {}
TRACE = False
PSUM2 = False  # tensor_mul with both operands in PSUM is illegal (NCC_IBVF027)


def kernel(x, Wq, Wkv, Wo, bo, Wo0, bo0):
    from concourse.bass_utils import run_bass_kernel_spmd

    x = np.asarray(x, dtype=np.float32)
    Wq = np.asarray(Wq, dtype=np.float32)
    Wkv = np.asarray(Wkv, dtype=np.float32)
    Wo = np.asarray(Wo, dtype=np.float32)
    bo = np.asarray(bo, dtype=np.float32)
    Wo0 = np.asarray(Wo0, dtype=np.float32)
    bo0 = np.asarray(bo0, dtype=np.float32)

    cosT, sinT, pshift, e2 = _host_tables()
    Wq_s = (Wq * SCALE).astype(np.float32)

    xTs = [np.ascontiguousarray(x[b_].T) for b_ in range(B)]
    in_maps = []
    for ci in range(NCORES):
        b_, hi = ci // 2, ci % 2
        fsl = slice(hi * FPC, (hi + 1) * FPC)
        in_maps.append(
            {
                "xT": xTs[b_],
                "wq": np.ascontiguousarray(Wq_s[:, fsl]),
                "wkv": np.ascontiguousarray(Wkv[:, fsl]),
                "wo": np.ascontiguousarray(Wo[fsl, :]),
                "wo0": np.ascontiguousarray(Wo0[fsl, :]),
                "cosT": cosT,
                "sinT": sinT,
                "pshift": pshift,
                "e2d": e2,
            }
        )

    if "nc" not in _CACHE:
        _CACHE["nc"] = _build_nc()
    nc = _CACHE["nc"]

    res = run_bass_kernel_spmd(
        nc, in_maps, core_ids=list(range(NCORES)), trace=TRACE
    )
    _CACHE["last"] = res
    parts = [r["outpT"] for r in res.results]

    out = np.empty((B, N, DIM), dtype=np.float32)
    bias = np.empty((N, DIM), dtype=np.float32)
    bias[:M] = bo0
    bias[M:] = bo
    for b_ in range(B):
        acc = parts[2 * b_] + parts[2 * b_ + 1]      # (768, 4096)
        out[b_] = acc.T + bias
    return out
